# revision 5
# baseline (speedup 1.0000x reference)
"""Causal self-attention Bass/Tile kernel for 8 Trainium2 NeuronCores (v3).

Problem: B=4, T=2048, C=1024, H=16, D=64 (fp32 in/out).
  qkv = x @ w_qkv + b_qkv ; causal softmax attention ; y @ w_out + b_out

Sharding (8 cores): core i handles batch b = i//2 and head-group hg = i%2
(8 of the 16 heads). Host sums the two partial output projections per batch
and adds b_out + b_v @ w_out (the V-bias commutes through attention).

v3 = v2 + software-pipelined phases: the QKV projection for T-chunk t+1 is
emitted interleaved with attention for q-chunks that only need chunks <= t,
so the Act engine's softmax-exp (the attention-phase bottleneck) overlaps
the PE-bound projection work instead of following it.

Key measured-on-HW facts this design is built on:
  - bf16 matmul: 1 col/cycle; fp32r: 2; M and K do not affect cost.
  - Two K=64 quadrant matmuls (tile_position (0,0)/(64,0)) stream
    concurrently: a head-pair's score tiles cost ~224 ns per 2x[128x512].
  - A PSUM bank must only ever be written by one PE tile geometry within a
    pool scope (mixing quadrant and full tiles on a bank faults the device).
  - Act exp costs ~0.83 ns/elem + ~260 ns/instruction: score tiles are
    [128,1024] (2 banks, one head-pair) so one exp covers both heads, and
    diagonal tiles exp only the live [d:512] columns.
  - The causal mask is a bf16 lower-triangle multiply on the DVE (post-exp)
    on the single 128-wide diagonal block.
  - Softmax denominators ride along as 64 ones-columns in the PV stationary
    ([ones64 | v64] per head, M=128): the denominator lands replicated in
    PSUM rows 0:64, so normalization is reciprocal_approx_fast + one
    tensor_mul: no partition_broadcast, no extra matmuls.
  - Weights are SBUF-resident in bf16; out-projection chunks are interleaved
    into later attention blocks to fill Act-gated PE bubbles; output is bf16.
"""

import numpy as np

B, T, C = 4, 2048, 1024
H, D = 16, 64
HL = 8          # heads per core
HP = HL // 2    # head-pairs per core
KCH = C // 128  # 8 contraction chunks
TCH = T // 512  # 4 T chunks of 512
SCALE = 1.0 / 8.0  # 1/sqrt(D)

_CACHE = {}


def _build():
    import concourse.bass as bass  # noqa: F401
    import concourse.mybir as mybir
    import concourse.tile as tile
    from concourse import bacc

    f32 = mybir.dt.float32
    bf16 = mybir.dt.bfloat16
    Exp = mybir.ActivationFunctionType.Exp
    Ident = mybir.ActivationFunctionType.Identity

    nc = bacc.Bacc("TRN2", target_bir_lowering=False, debug=False, num_devices=8)

    xt_d = nc.dram_tensor("xt", [C, T], bf16, kind="ExternalInput")
    wqk_d = nc.dram_tensor("wqk", [C, 1024], bf16, kind="ExternalInput")
    wv_d = nc.dram_tensor("wv", [C, 512], bf16, kind="ExternalInput")
    wo_d = nc.dram_tensor("wo", [512, 1024], bf16, kind="ExternalInput")
    bqk_d = nc.dram_tensor("bqk", [128, 8], f32, kind="ExternalInput")
    out_d = nc.dram_tensor("outT", [1024, T], bf16, kind="ExternalOutput")

    with tile.TileContext(nc) as tc, nc.allow_low_precision(
        reason="bf16 matmul pipeline; rel error budget 2e-2"
    ):
        with (
            tc.tile_pool(name="wq", bufs=1) as w_pool,
            tc.tile_pool(name="qt", bufs=HP) as qt_pool,
            tc.tile_pool(name="kt", bufs=HP) as kt_pool,
            tc.tile_pool(name="v", bufs=16) as v_pool,
            tc.tile_pool(name="yt", bufs=HP) as yt_pool,
            tc.tile_pool(name="p", bufs=4) as p_pool,
            tc.tile_pool(name="misc", bufs=1) as misc_pool,
            tc.tile_pool(name="rcp", bufs=4) as rcp_pool,
            tc.tile_pool(name="xt", bufs=16) as xt_pool,
            tc.tile_pool(name="ostage", bufs=3) as ostage_pool,
            tc.tile_pool(name="ps_y", bufs=1, space="PSUM") as ps_y,
        ):
            # ---- resident weights ----
            wqk_sb = [w_pool.tile([128, 1024], bf16, tag=f"wqk{k}", name=f"wqk{k}")
                      for k in range(KCH)]
            wv_sb = [w_pool.tile([128, 512], bf16, tag=f"wv{k}", name=f"wv{k}")
                     for k in range(KCH)]
            wo_sb = [w_pool.tile([128, 1024], bf16, tag=f"wo{k}", name=f"wo{k}")
                     for k in range(4)]
            bqk_sb = misc_pool.tile([128, 8], f32, tag="bqk", name="bqk")
            nc.sync.dma_start(out=bqk_sb[:], in_=bqk_d[:])

            # lower-triangle (keep j >= p) bf16 mask, two copies side by side
            tri = misc_pool.tile([128, 256], bf16, tag="tri", name="tri")
            nc.vector.memset(tri[:], 1.0)
            for half in range(2):
                nc.gpsimd.affine_select(
                    out=tri[:, half * 128:(half + 1) * 128],
                    in_=tri[:, half * 128:(half + 1) * 128],
                    compare_op=mybir.AluOpType.is_ge,
                    fill=0.0, base=0, pattern=[[1, 128]], channel_multiplier=-1)

            qt = [qt_pool.tile([128, T], bf16, tag="qt", name="qt") for _ in range(HP)]
            kt = [kt_pool.tile([128, T], bf16, tag="kt", name="kt") for _ in range(HP)]
            v_sb = [v_pool.tile([128, 1024], bf16, tag="v", name="v") for _ in range(16)]
            yt = [yt_pool.tile([128, T], bf16, tag="yt", name="yt") for _ in range(HP)]

            xtc_all = {}

            def emit_xtc_dma(tch):
                ts = tch * 512
                xtc = [xt_pool.tile([128, 512], bf16, tag="xt", name="xt")
                       for _ in range(KCH)]
                xtc_all[tch] = xtc
                if tch == 0:
                    # critical path of the first matmul: xtc[0] + wqk[0]
                    nc.sync.dma_start(out=xtc[0][:], in_=xt_d[0:128, 0:512])
                    for qq in range(4):
                        nc.sync.dma_start(
                            out=wqk_sb[0][:, qq * 256:(qq + 1) * 256],
                            in_=wqk_d[0:128, qq * 256:(qq + 1) * 256])
                for k in range(KCH):
                    if tch == 0 and k == 0:
                        continue
                    nc.sync.dma_start(
                        out=xtc[k][:],
                        in_=xt_d[k * 128:(k + 1) * 128, ts:ts + 512])
                if tch == 0:
                    # weight DMAs issue from the (early-idle) Act queue in
                    # parallel with the sync queue's xtc stream
                    for k in range(1, KCH):
                        for hh in range(2):
                            nc.scalar.dma_start(
                                out=wqk_sb[k][:, hh * 512:(hh + 1) * 512],
                                in_=wqk_d[k * 128:(k + 1) * 128,
                                          hh * 512:(hh + 1) * 512])
                    for k in range(KCH):
                        nc.sync.dma_start(
                            out=wv_sb[k][:], in_=wv_d[k * 128:(k + 1) * 128, :])
                if tch == 1:
                    for k in range(4):
                        nc.sync.dma_start(
                            out=wo_sb[k][:], in_=wo_d[k * 128:(k + 1) * 128, :])

            def qk_chain(pj_pool, tch, ct):
                ts = tch * 512
                xtc = xtc_all[tch]
                pj = pj_pool.tile([128, 512], f32, tag="pj", name="pj")
                for k in range(KCH):
                    nc.tensor.matmul(
                        pj[:], wqk_sb[k][:, ct * 128:(ct + 1) * 128], xtc[k][:],
                        start=(k == 0), stop=(k == KCH - 1))
                dst = qt[ct] if ct < HP else kt[ct - HP]
                if ct % 2 == 0:
                    nc.scalar.activation(dst[:, ts:ts + 512], pj[:], Ident,
                                         bias=bqk_sb[:, ct:ct + 1])
                else:
                    nc.vector.tensor_scalar_add(dst[:, ts:ts + 512], pj[:],
                                                bqk_sb[:, ct:ct + 1])

            def v_chain(pj_pool, tch, tl):
                xtc = xtc_all[tch]
                pj = pj_pool.tile([128, 512], f32, tag="pj", name="pj")
                for k in range(KCH):
                    nc.tensor.matmul(
                        pj[:], xtc[k][:, tl * 128:(tl + 1) * 128], wv_sb[k][:],
                        start=(k == 0), stop=(k == KCH - 1))
                tt = tch * 4 + tl
                vt = v_sb[tt]
                v_view = vt[:].rearrange("p (h c) -> p h c", c=128)
                srcv = pj[:].rearrange("p (h c) -> p h c", c=64)
                if tl % 2 == 0:
                    nc.scalar.activation(v_view[:, :, 64:128], srcv, Ident)
                else:
                    nc.vector.tensor_copy(v_view[:, :, 64:128], srcv)
                nc.gpsimd.memset(v_view[:, :, 0:64], 1.0)

            def attn_block(s_pool, qc, hp):
                """Generator: one (qc, hp) attention block, yielding after
                each kti unit so the driver can interleave other PE work."""
                qs = qc * 512
                ya = ps_y.tile([128, 512], f32, tag="ya", name="ya")
                yb = ps_y.tile([128, 512], f32, tag="yb", name="yb")
                emit = [4 * qc + j for j in range(4)] + list(range(4 * qc))

                def flush_pv(kti, p_t, d, ia, ib):
                    w0 = 0 if qc == 0 else max(d, 0)
                    ha, hb = 2 * hp, 2 * hp + 1
                    nc.tensor.matmul(
                        ya[:, w0:512],
                        v_sb[kti][:, ha * 128:(ha + 1) * 128],
                        p_t[:, w0:512], start=ia, stop=ib)
                    nc.tensor.matmul(
                        yb[:, w0:512],
                        v_sb[kti][:, hb * 128:(hb + 1) * 128],
                        p_t[:, 512 + w0:1024], start=ia, stop=ib)

                pend = []
                for kti in emit:
                    ks = kti * 128
                    d = ks - qs
                    w0 = max(d, 0)
                    s_t = s_pool.tile([128, 1024], f32, tag="s", name="s")
                    nc.tensor.matmul(
                        s_t[:, w0:512],
                        kt[hp][0:64, ks:ks + 128],
                        qt[hp][0:64, qs + w0:qs + 512],
                        start=True, stop=True, tile_position=(0, 0))
                    nc.tensor.matmul(
                        s_t[:, 512 + w0:1024],
                        kt[hp][64:128, ks:ks + 128],
                        qt[hp][64:128, qs + w0:qs + 512],
                        start=True, stop=True, tile_position=(64, 0))
                    p_t = p_pool.tile([128, 1024], bf16, tag="p", name="p")
                    sv = s_t[:].rearrange("p (b c) -> p b c", c=512)
                    pv = p_t[:].rearrange("p (b c) -> p b c", c=512)
                    nc.scalar.activation(pv[:, :, w0:512], sv[:, :, w0:512],
                                         Exp, scale=SCALE)
                    if d >= 0:
                        nc.vector.tensor_mul(
                            pv[:, :, d:d + 128], pv[:, :, d:d + 128],
                            tri[:].rearrange("p (b c) -> p b c", c=128))
                        if qc == 0 and w0 > 0:
                            nc.gpsimd.memset(pv[:, :, 0:w0], 0.0)
                    pend.append((kti, p_t, d))
                    if len(pend) > 1:
                        k0, p0, d0 = pend.pop(0)
                        flush_pv(k0, p0, d0, k0 == emit[0], False)
                    yield
                k0, p0, d0 = pend.pop(0)
                flush_pv(k0, p0, d0, k0 == emit[0], True)
                # normalize: y rows 64:128 / replicated denominator rows 0:64
                for off, yy in ((0, ya), (64, yb)):
                    rcp = rcp_pool.tile([64, 512], f32, tag="rcp", name="rcp")
                    nc.vector.reciprocal_approx_fast(out=rcp[:], in_=yy[0:64, :])
                    nc.vector.tensor_mul(
                        yt[hp][off:off + 64, qs:qs + 512],
                        yy[64:128, :], rcp[:])
                yield

            def outproj_chunk(po_pool, qc_, ct, split_dma=False):
                qs_ = qc_ * 512
                po = po_pool.tile([128, 512], f32, tag="po", name="po")
                for k in range(4):
                    nc.tensor.matmul(
                        po[:], wo_sb[k][:, ct * 128:(ct + 1) * 128],
                        yt[k][:, qs_:qs_ + 512],
                        start=(k == 0), stop=(k == 3))
                st = ostage_pool.tile([128, 512], bf16, tag="ost", name="ost")
                nc.vector.tensor_copy(st[:], po[:])
                if split_dma:
                    # tail chunks: halve per-queue transfer time
                    for hh in range(2):
                        nc.sync.dma_start(
                            out=out_d[ct * 128:(ct + 1) * 128,
                                      qs_ + hh * 256:qs_ + (hh + 1) * 256],
                            in_=st[:, hh * 256:(hh + 1) * 256])
                else:
                    nc.sync.dma_start(
                        out=out_d[ct * 128:(ct + 1) * 128, qs_:qs_ + 512],
                        in_=st[:])

            def drain(g):
                for _ in g:
                    pass

            # ====== scope 0: tch0 projection, k-outer across 6 banks ======
            # (k-inner chains would stall on weight-DMA arrival order here;
            # k-outer matches the DMA issue order so PE streams immediately)
            emit_xtc_dma(0)
            emit_xtc_dma(1)
            with tc.tile_pool(name="pj0", bufs=6, space="PSUM") as pj0:
                xtc = xtc_all[0]
                pja = [pj0.tile([128, 512], f32, tag="pj", name="pj")
                       for _ in range(6)]
                for k in range(KCH):
                    for ct in range(6):
                        nc.tensor.matmul(
                            pja[ct][:], wqk_sb[k][:, ct * 128:(ct + 1) * 128],
                            xtc[k][:], start=(k == 0), stop=(k == KCH - 1))
                for ct in range(6):
                    dst = qt[ct] if ct < HP else kt[ct - HP]
                    nc.vector.tensor_scalar_add(dst[:, 0:512], pja[ct][:],
                                                bqk_sb[:, ct:ct + 1])
                pjb = [pj0.tile([128, 512], f32, tag="pj", name="pj")
                       for _ in range(2)]
                for k in range(KCH):
                    for i, ct in enumerate((6, 7)):
                        nc.tensor.matmul(
                            pjb[i][:], wqk_sb[k][:, ct * 128:(ct + 1) * 128],
                            xtc[k][:], start=(k == 0), stop=(k == KCH - 1))
                for i, ct in enumerate((6, 7)):
                    dst = kt[ct - HP]
                    nc.vector.tensor_scalar_add(dst[:, 0:512], pjb[i][:],
                                                bqk_sb[:, ct:ct + 1])
                pjv = [pj0.tile([128, 512], f32, tag="pj", name="pj")
                       for _ in range(4)]
                for k in range(KCH):
                    for tl in range(4):
                        nc.tensor.matmul(
                            pjv[tl][:], xtc[k][:, tl * 128:(tl + 1) * 128],
                            wv_sb[k][:], start=(k == 0), stop=(k == KCH - 1))
                for tl in range(4):
                    vt = v_sb[tl]
                    v_view = vt[:].rearrange("p (h c) -> p h c", c=128)
                    srcv = pjv[tl][:].rearrange("p (h c) -> p h c", c=64)
                    if tl % 2 == 0:
                        nc.scalar.activation(v_view[:, :, 64:128], srcv, Ident)
                    else:
                        nc.vector.tensor_copy(v_view[:, :, 64:128], srcv)
                    nc.gpsimd.memset(v_view[:, :, 0:64], 1.0)

            # ================= scope 1: tch1-3 chains + qc0..qc2(hp0,1) =====
            with (
                tc.tile_pool(name="pj", bufs=2, space="PSUM") as pj_pool,
                tc.tile_pool(name="s1", bufs=2, space="PSUM") as s1_pool,
            ):

                # attention unit streams, eligible per completed tch
                streams = []
                for qc, hps in ((0, range(HP)), (1, range(HP)), (2, (0, 1))):
                    for hp in hps:
                        streams.append((qc, attn_block(s1_pool, qc, hp)))
                si = 0          # index into streams
                cur = None

                def next_unit(max_qc):
                    nonlocal si, cur
                    while si < len(streams):
                        qc, g = streams[si]
                        if qc > max_qc:
                            return False
                        try:
                            next(g)
                            return True
                        except StopIteration:
                            si += 1
                    return False

                # tch1 chains <-> qc0 units
                emit_xtc_dma(2)
                for ci, ct in enumerate(range(8)):
                    qk_chain(pj_pool, 1, ct)
                    next_unit(0)
                    if ci % 2 == 0:
                        next_unit(0)
                for tl in range(4):
                    v_chain(pj_pool, 1, tl)
                    next_unit(0)
                # tch2 chains <-> qc1 units
                emit_xtc_dma(3)
                for ct in range(8):
                    qk_chain(pj_pool, 2, ct)
                    next_unit(1)
                    next_unit(1)
                for tl in range(4):
                    v_chain(pj_pool, 2, tl)
                    next_unit(1)
                    next_unit(1)
                # tch3 chains <-> qc1 rest + qc2(hp0,1) units
                for ct in range(8):
                    qk_chain(pj_pool, 3, ct)
                    next_unit(2)
                    next_unit(2)
                    next_unit(2)
                for tl in range(4):
                    v_chain(pj_pool, 3, tl)
                    next_unit(2)
                    next_unit(2)
                    next_unit(2)
                # finish remaining scope-1 attention
                while next_unit(2):
                    pass

            # ===== scope 2: qc2(hp2,3) + qc3 + all output projection ========
            with (
                tc.tile_pool(name="s2", bufs=2, space="PSUM") as s2_pool,
                tc.tile_pool(name="ps_o", bufs=2, space="PSUM") as po_pool,
            ):
                # qc2 hp2/hp3 with qc0's out-proj interleaved
                for i, hp in enumerate((2, 3)):
                    drain(attn_block(s2_pool, 2, hp))
                    for ct in range(4 * i, 4 * i + 4):
                        outproj_chunk(po_pool, 0, ct)
                # qc3 with qc1/qc2 out-proj interleaved
                for hp in range(HP):
                    drain(attn_block(s2_pool, 3, hp))
                    for qq, ct in ((1, 2 * hp), (1, 2 * hp + 1),
                                   (2, 2 * hp), (2, 2 * hp + 1)):
                        outproj_chunk(po_pool, qq, ct)
                for ct in range(8):
                    outproj_chunk(po_pool, 3, ct, split_dma=(ct >= 4))

    nc.compile()
    return nc


def _get_nc():
    if "nc" not in _CACHE:
        _CACHE["nc"] = _build()
    return _CACHE["nc"]


def kernel(x, w_qkv, b_qkv, w_out, b_out):
    import ml_dtypes
    from concourse.bass_utils import run_bass_kernel_spmd

    bf = ml_dtypes.bfloat16
    x = np.asarray(x, dtype=np.float32)
    w_qkv = np.asarray(w_qkv, dtype=np.float32)
    b_qkv = np.asarray(b_qkv, dtype=np.float32)
    w_out = np.asarray(w_out, dtype=np.float32)
    b_out = np.asarray(b_out, dtype=np.float32)

    in_maps = []
    for core in range(8):
        b = core // 2
        hg = core % 2
        cs = hg * 512
        wqk = np.empty((C, 1024), dtype=bf)
        wqk[:, 0:512] = w_qkv[:, cs:cs + 512]
        wqk[:, 512:1024] = w_qkv[:, C + cs:C + cs + 512]
        bqk = np.empty((128, 8), dtype=np.float32)
        for j in range(4):
            bqk[:, j] = b_qkv[cs + j * 128: cs + (j + 1) * 128]
            bqk[:, 4 + j] = b_qkv[C + cs + j * 128: C + cs + (j + 1) * 128]
        in_maps.append({
            "xt": np.ascontiguousarray(x[b].T).astype(bf),
            "wqk": wqk,
            "wv": w_qkv[:, 2 * C + cs:2 * C + cs + 512].astype(bf),
            "wo": np.ascontiguousarray(w_out[cs:cs + 512, :]).astype(bf),
            "bqk": bqk,
        })

    _CACHE["in_maps"] = in_maps
    res = run_bass_kernel_spmd(_get_nc(), in_maps, core_ids=list(range(8)))

    # host epilogue: sum head-group partials, add b_out and the V-bias term
    b_eff = b_out + b_qkv[2 * C:3 * C] @ w_out
    out = np.empty((B, T, C), dtype=np.float32)
    for b in range(B):
        acc = (res.results[2 * b]["outT"].astype(np.float32)
               + res.results[2 * b + 1]["outT"].astype(np.float32))
        out[b] = acc.T + b_eff[None, :]
    return out


# revision 6
# speedup vs baseline: 1.0020x; 1.0020x over previous
"""Causal self-attention Bass/Tile kernel for 8 Trainium2 NeuronCores (v3).

Problem: B=4, T=2048, C=1024, H=16, D=64 (fp32 in/out).
  qkv = x @ w_qkv + b_qkv ; causal softmax attention ; y @ w_out + b_out

Sharding (8 cores): core i handles batch b = i//2 and head-group hg = i%2
(8 of the 16 heads). Host sums the two partial output projections per batch
and adds b_out + b_v @ w_out (the V-bias commutes through attention).

v3 = v2 + software-pipelined phases: the QKV projection for T-chunk t+1 is
emitted interleaved with attention for q-chunks that only need chunks <= t,
so the Act engine's softmax-exp (the attention-phase bottleneck) overlaps
the PE-bound projection work instead of following it.

Key measured-on-HW facts this design is built on:
  - bf16 matmul: 1 col/cycle; fp32r: 2; M and K do not affect cost.
  - Two K=64 quadrant matmuls (tile_position (0,0)/(64,0)) stream
    concurrently: a head-pair's score tiles cost ~224 ns per 2x[128x512].
  - A PSUM bank must only ever be written by one PE tile geometry within a
    pool scope (mixing quadrant and full tiles on a bank faults the device).
  - Act exp costs ~0.83 ns/elem + ~260 ns/instruction: score tiles are
    [128,1024] (2 banks, one head-pair) so one exp covers both heads, and
    diagonal tiles exp only the live [d:512] columns.
  - The causal mask is a bf16 lower-triangle multiply on the DVE (post-exp)
    on the single 128-wide diagonal block.
  - Softmax denominators ride along as 64 ones-columns in the PV stationary
    ([ones64 | v64] per head, M=128): the denominator lands replicated in
    PSUM rows 0:64, so normalization is reciprocal_approx_fast + one
    tensor_mul: no partition_broadcast, no extra matmuls.
  - Weights are SBUF-resident in bf16; out-projection chunks are interleaved
    into later attention blocks to fill Act-gated PE bubbles; output is bf16.
"""

import numpy as np

B, T, C = 4, 2048, 1024
H, D = 16, 64
HL = 8          # heads per core
HP = HL // 2    # head-pairs per core
KCH = C // 128  # 8 contraction chunks
TCH = T // 512  # 4 T chunks of 512
SCALE = 1.0 / 8.0  # 1/sqrt(D)

_CACHE = {}


def _build():
    import concourse.bass as bass  # noqa: F401
    import concourse.mybir as mybir
    import concourse.tile as tile
    from concourse import bacc

    f32 = mybir.dt.float32
    bf16 = mybir.dt.bfloat16
    Exp = mybir.ActivationFunctionType.Exp
    Ident = mybir.ActivationFunctionType.Identity

    nc = bacc.Bacc("TRN2", target_bir_lowering=False, debug=False, num_devices=8)

    xt_d = nc.dram_tensor("xt", [C, T], bf16, kind="ExternalInput")
    wqk_d = nc.dram_tensor("wqk", [C, 1024], bf16, kind="ExternalInput")
    wv_d = nc.dram_tensor("wv", [C, 512], bf16, kind="ExternalInput")
    wo_d = nc.dram_tensor("wo", [512, 1024], bf16, kind="ExternalInput")
    bqk_d = nc.dram_tensor("bqk", [128, 8], f32, kind="ExternalInput")
    out_d = nc.dram_tensor("outT", [1024, T], bf16, kind="ExternalOutput")

    with tile.TileContext(nc) as tc, nc.allow_low_precision(
        reason="bf16 matmul pipeline; rel error budget 2e-2"
    ):
        with (
            tc.tile_pool(name="wq", bufs=1) as w_pool,
            tc.tile_pool(name="qt", bufs=HP) as qt_pool,
            tc.tile_pool(name="kt", bufs=HP) as kt_pool,
            tc.tile_pool(name="v", bufs=16) as v_pool,
            tc.tile_pool(name="yt", bufs=HP) as yt_pool,
            tc.tile_pool(name="p", bufs=4) as p_pool,
            tc.tile_pool(name="misc", bufs=1) as misc_pool,
            tc.tile_pool(name="rcp", bufs=4) as rcp_pool,
            tc.tile_pool(name="xt", bufs=16) as xt_pool,
            tc.tile_pool(name="ostage", bufs=3) as ostage_pool,
            tc.tile_pool(name="ps_y", bufs=1, space="PSUM") as ps_y,
        ):
            # ---- resident weights ----
            wqk_sb = [w_pool.tile([128, 1024], bf16, tag=f"wqk{k}", name=f"wqk{k}")
                      for k in range(KCH)]
            wv_sb = [w_pool.tile([128, 512], bf16, tag=f"wv{k}", name=f"wv{k}")
                     for k in range(KCH)]
            wo_sb = [w_pool.tile([128, 1024], bf16, tag=f"wo{k}", name=f"wo{k}")
                     for k in range(4)]
            bqk_sb = misc_pool.tile([128, 8], f32, tag="bqk", name="bqk")
            nc.sync.dma_start(out=bqk_sb[:], in_=bqk_d[:])

            # lower-triangle (keep j >= p) bf16 mask, two copies side by side
            tri = misc_pool.tile([128, 256], bf16, tag="tri", name="tri")
            nc.vector.memset(tri[:], 1.0)
            for half in range(2):
                nc.gpsimd.affine_select(
                    out=tri[:, half * 128:(half + 1) * 128],
                    in_=tri[:, half * 128:(half + 1) * 128],
                    compare_op=mybir.AluOpType.is_ge,
                    fill=0.0, base=0, pattern=[[1, 128]], channel_multiplier=-1)

            qt = [qt_pool.tile([128, T], bf16, tag="qt", name="qt") for _ in range(HP)]
            kt = [kt_pool.tile([128, T], bf16, tag="kt", name="kt") for _ in range(HP)]
            v_sb = [v_pool.tile([128, 1024], bf16, tag="v", name="v") for _ in range(16)]
            yt = [yt_pool.tile([128, T], bf16, tag="yt", name="yt") for _ in range(HP)]

            xtc_all = {}

            def emit_xtc_dma(tch):
                ts = tch * 512
                xtc = [xt_pool.tile([128, 512], bf16, tag="xt", name="xt")
                       for _ in range(KCH)]
                xtc_all[tch] = xtc
                if tch == 0:
                    # critical path of the first matmul: xtc[0] + wqk[0]
                    nc.sync.dma_start(out=xtc[0][:], in_=xt_d[0:128, 0:512])
                    for qq in range(4):
                        nc.sync.dma_start(
                            out=wqk_sb[0][:, qq * 256:(qq + 1) * 256],
                            in_=wqk_d[0:128, qq * 256:(qq + 1) * 256])
                for k in range(KCH):
                    if tch == 0 and k == 0:
                        continue
                    nc.sync.dma_start(
                        out=xtc[k][:],
                        in_=xt_d[k * 128:(k + 1) * 128, ts:ts + 512])
                if tch == 0:
                    # weight DMAs issue from the (early-idle) Act queue in
                    # parallel with the sync queue's xtc stream
                    for k in range(1, KCH):
                        for hh in range(2):
                            nc.scalar.dma_start(
                                out=wqk_sb[k][:, hh * 512:(hh + 1) * 512],
                                in_=wqk_d[k * 128:(k + 1) * 128,
                                          hh * 512:(hh + 1) * 512])
                    for k in range(KCH):
                        nc.sync.dma_start(
                            out=wv_sb[k][:], in_=wv_d[k * 128:(k + 1) * 128, :])
                if tch == 1:
                    for k in range(4):
                        nc.sync.dma_start(
                            out=wo_sb[k][:], in_=wo_d[k * 128:(k + 1) * 128, :])

            def qk_chain(pj_pool, tch, ct):
                ts = tch * 512
                xtc = xtc_all[tch]
                pj = pj_pool.tile([128, 512], f32, tag="pj", name="pj")
                for k in range(KCH):
                    nc.tensor.matmul(
                        pj[:], wqk_sb[k][:, ct * 128:(ct + 1) * 128], xtc[k][:],
                        start=(k == 0), stop=(k == KCH - 1))
                dst = qt[ct] if ct < HP else kt[ct - HP]
                if ct % 2 == 0:
                    nc.scalar.activation(dst[:, ts:ts + 512], pj[:], Ident,
                                         bias=bqk_sb[:, ct:ct + 1])
                else:
                    nc.vector.tensor_scalar_add(dst[:, ts:ts + 512], pj[:],
                                                bqk_sb[:, ct:ct + 1])

            def v_chain(pj_pool, tch, tl):
                xtc = xtc_all[tch]
                pj = pj_pool.tile([128, 512], f32, tag="pj", name="pj")
                for k in range(KCH):
                    nc.tensor.matmul(
                        pj[:], xtc[k][:, tl * 128:(tl + 1) * 128], wv_sb[k][:],
                        start=(k == 0), stop=(k == KCH - 1))
                tt = tch * 4 + tl
                vt = v_sb[tt]
                v_view = vt[:].rearrange("p (h c) -> p h c", c=128)
                srcv = pj[:].rearrange("p (h c) -> p h c", c=64)
                if tl % 2 == 0:
                    nc.scalar.activation(v_view[:, :, 64:128], srcv, Ident)
                else:
                    nc.vector.tensor_copy(v_view[:, :, 64:128], srcv)
                nc.gpsimd.memset(v_view[:, :, 0:64], 1.0)

            def attn_block(s_pool, qc, hp):
                """Generator: one (qc, hp) attention block, yielding after
                each kti unit so the driver can interleave other PE work."""
                qs = qc * 512
                ya = ps_y.tile([128, 512], f32, tag="ya", name="ya")
                yb = ps_y.tile([128, 512], f32, tag="yb", name="yb")
                emit = [4 * qc + j for j in range(4)] + list(range(4 * qc))

                def flush_pv(kti, p_t, d, ia, ib):
                    w0 = 0 if qc == 0 else max(d, 0)
                    ha, hb = 2 * hp, 2 * hp + 1
                    nc.tensor.matmul(
                        ya[:, w0:512],
                        v_sb[kti][:, ha * 128:(ha + 1) * 128],
                        p_t[:, w0:512], start=ia, stop=ib)
                    nc.tensor.matmul(
                        yb[:, w0:512],
                        v_sb[kti][:, hb * 128:(hb + 1) * 128],
                        p_t[:, 512 + w0:1024], start=ia, stop=ib)

                pend = []
                for kti in emit:
                    ks = kti * 128
                    d = ks - qs
                    w0 = max(d, 0)
                    s_t = s_pool.tile([128, 1024], f32, tag="s", name="s")
                    nc.tensor.matmul(
                        s_t[:, w0:512],
                        kt[hp][0:64, ks:ks + 128],
                        qt[hp][0:64, qs + w0:qs + 512],
                        start=True, stop=True, tile_position=(0, 0))
                    nc.tensor.matmul(
                        s_t[:, 512 + w0:1024],
                        kt[hp][64:128, ks:ks + 128],
                        qt[hp][64:128, qs + w0:qs + 512],
                        start=True, stop=True, tile_position=(64, 0))
                    p_t = p_pool.tile([128, 1024], bf16, tag="p", name="p")
                    sv = s_t[:].rearrange("p (b c) -> p b c", c=512)
                    pv = p_t[:].rearrange("p (b c) -> p b c", c=512)
                    nc.scalar.activation(pv[:, :, w0:512], sv[:, :, w0:512],
                                         Exp, scale=SCALE)
                    if d >= 0:
                        nc.vector.tensor_mul(
                            pv[:, :, d:d + 128], pv[:, :, d:d + 128],
                            tri[:].rearrange("p (b c) -> p b c", c=128))
                        if qc == 0 and w0 > 0:
                            nc.gpsimd.memset(pv[:, :, 0:w0], 0.0)
                    pend.append((kti, p_t, d))
                    if len(pend) > 1:
                        k0, p0, d0 = pend.pop(0)
                        flush_pv(k0, p0, d0, k0 == emit[0], False)
                    yield
                k0, p0, d0 = pend.pop(0)
                flush_pv(k0, p0, d0, k0 == emit[0], True)
                # normalize: y rows 64:128 / replicated denominator rows 0:64
                for off, yy in ((0, ya), (64, yb)):
                    rcp = rcp_pool.tile([64, 512], f32, tag="rcp", name="rcp")
                    nc.vector.reciprocal_approx_fast(out=rcp[:], in_=yy[0:64, :])
                    nc.vector.tensor_mul(
                        yt[hp][off:off + 64, qs:qs + 512],
                        yy[64:128, :], rcp[:])
                yield

            def outproj_chunk(po_pool, qc_, ct, split_dma=False):
                qs_ = qc_ * 512
                po = po_pool.tile([128, 512], f32, tag="po", name="po")
                for k in range(4):
                    nc.tensor.matmul(
                        po[:], wo_sb[k][:, ct * 128:(ct + 1) * 128],
                        yt[k][:, qs_:qs_ + 512],
                        start=(k == 0), stop=(k == 3))
                st = ostage_pool.tile([128, 512], bf16, tag="ost", name="ost")
                nc.vector.tensor_copy(st[:], po[:])
                if split_dma:
                    # tail chunks: halve per-queue transfer time
                    for hh in range(2):
                        nc.sync.dma_start(
                            out=out_d[ct * 128:(ct + 1) * 128,
                                      qs_ + hh * 256:qs_ + (hh + 1) * 256],
                            in_=st[:, hh * 256:(hh + 1) * 256])
                else:
                    nc.sync.dma_start(
                        out=out_d[ct * 128:(ct + 1) * 128, qs_:qs_ + 512],
                        in_=st[:])

            def outproj_half(po_pool, qc_, ct, hh):
                qs_ = qc_ * 512 + hh * 256
                pof = po_pool.tile([128, 512], f32, tag="po", name="poh")
                po = pof[:, 0:256]
                for k in range(4):
                    nc.tensor.matmul(
                        po, wo_sb[k][:, ct * 128:(ct + 1) * 128],
                        yt[k][:, qs_:qs_ + 256],
                        start=(k == 0), stop=(k == 3))
                st = ostage_pool.tile([128, 256], bf16, tag="osth", name="osth")
                nc.vector.tensor_copy(st[:], po)
                nc.sync.dma_start(
                    out=out_d[ct * 128:(ct + 1) * 128, qs_:qs_ + 256],
                    in_=st[:])

            def drain(g):
                for _ in g:
                    pass

            # ====== scope 0: tch0 projection, k-outer across 6 banks ======
            # (k-inner chains would stall on weight-DMA arrival order here;
            # k-outer matches the DMA issue order so PE streams immediately)
            emit_xtc_dma(0)
            emit_xtc_dma(1)
            with tc.tile_pool(name="pj0", bufs=6, space="PSUM") as pj0:
                xtc = xtc_all[0]
                pja = [pj0.tile([128, 512], f32, tag="pj", name="pj")
                       for _ in range(6)]
                for k in range(KCH):
                    for ct in range(6):
                        nc.tensor.matmul(
                            pja[ct][:], wqk_sb[k][:, ct * 128:(ct + 1) * 128],
                            xtc[k][:], start=(k == 0), stop=(k == KCH - 1))
                for ct in range(6):
                    dst = qt[ct] if ct < HP else kt[ct - HP]
                    nc.vector.tensor_scalar_add(dst[:, 0:512], pja[ct][:],
                                                bqk_sb[:, ct:ct + 1])
                pjb = [pj0.tile([128, 512], f32, tag="pj", name="pj")
                       for _ in range(2)]
                for k in range(KCH):
                    for i, ct in enumerate((6, 7)):
                        nc.tensor.matmul(
                            pjb[i][:], wqk_sb[k][:, ct * 128:(ct + 1) * 128],
                            xtc[k][:], start=(k == 0), stop=(k == KCH - 1))
                for i, ct in enumerate((6, 7)):
                    dst = kt[ct - HP]
                    nc.vector.tensor_scalar_add(dst[:, 0:512], pjb[i][:],
                                                bqk_sb[:, ct:ct + 1])
                pjv = [pj0.tile([128, 512], f32, tag="pj", name="pj")
                       for _ in range(4)]
                for k in range(KCH):
                    for tl in range(4):
                        nc.tensor.matmul(
                            pjv[tl][:], xtc[k][:, tl * 128:(tl + 1) * 128],
                            wv_sb[k][:], start=(k == 0), stop=(k == KCH - 1))
                for tl in range(4):
                    vt = v_sb[tl]
                    v_view = vt[:].rearrange("p (h c) -> p h c", c=128)
                    srcv = pjv[tl][:].rearrange("p (h c) -> p h c", c=64)
                    if tl % 2 == 0:
                        nc.scalar.activation(v_view[:, :, 64:128], srcv, Ident)
                    else:
                        nc.vector.tensor_copy(v_view[:, :, 64:128], srcv)
                    nc.gpsimd.memset(v_view[:, :, 0:64], 1.0)

            # ================= scope 1: tch1-3 chains + qc0..qc2(hp0,1) =====
            with (
                tc.tile_pool(name="pj", bufs=2, space="PSUM") as pj_pool,
                tc.tile_pool(name="s1", bufs=2, space="PSUM") as s1_pool,
            ):

                # attention unit streams, eligible per completed tch
                streams = []
                for qc, hps in ((0, range(HP)), (1, range(HP)), (2, (0, 1))):
                    for hp in hps:
                        streams.append((qc, attn_block(s1_pool, qc, hp)))
                si = 0          # index into streams
                cur = None

                def next_unit(max_qc):
                    nonlocal si, cur
                    while si < len(streams):
                        qc, g = streams[si]
                        if qc > max_qc:
                            return False
                        try:
                            next(g)
                            return True
                        except StopIteration:
                            si += 1
                    return False

                # tch1 chains <-> qc0 units
                emit_xtc_dma(2)
                for ci, ct in enumerate(range(8)):
                    qk_chain(pj_pool, 1, ct)
                    next_unit(0)
                    if ci % 2 == 0:
                        next_unit(0)
                for tl in range(4):
                    v_chain(pj_pool, 1, tl)
                    next_unit(0)
                # tch2 chains <-> qc1 units
                emit_xtc_dma(3)
                for ct in range(8):
                    qk_chain(pj_pool, 2, ct)
                    next_unit(1)
                    next_unit(1)
                for tl in range(4):
                    v_chain(pj_pool, 2, tl)
                    next_unit(1)
                    next_unit(1)
                # tch3 chains <-> qc1 rest + qc2(hp0,1) units
                for ct in range(8):
                    qk_chain(pj_pool, 3, ct)
                    next_unit(2)
                    next_unit(2)
                    next_unit(2)
                for tl in range(4):
                    v_chain(pj_pool, 3, tl)
                    next_unit(2)
                    next_unit(2)
                    next_unit(2)
                # finish remaining scope-1 attention
                while next_unit(2):
                    pass

            # ===== scope 2: qc2(hp2,3) + qc3 + all output projection ========
            with (
                tc.tile_pool(name="s2", bufs=2, space="PSUM") as s2_pool,
                tc.tile_pool(name="ps_o", bufs=2, space="PSUM") as po_pool,
            ):
                # qc2 hp2/hp3 with qc0's out-proj interleaved (3 chunks
                # each; one is deferred to the Act-bound late qc3 blocks)
                for i, hp in enumerate((2, 3)):
                    drain(attn_block(s2_pool, 2, hp))
                    for ct in range(4 * i, 4 * i + 3):
                        outproj_chunk(po_pool, 0, ct)
                # qc3 with qc1/qc2 out-proj interleaved
                for hp in range(HP):
                    drain(attn_block(s2_pool, 3, hp))
                    for qq, ct in ((1, 2 * hp), (1, 2 * hp + 1),
                                   (2, 2 * hp), (2, 2 * hp + 1)):
                        outproj_chunk(po_pool, qq, ct)
                    if hp == 2:
                        outproj_chunk(po_pool, 0, 3)
                    if hp == 3:
                        outproj_chunk(po_pool, 0, 7)
                # last q-chunk: first 6 full, last 2 as half-width pieces so
                # the serial matmul->copy->DMA tail is shorter
                for ct in range(6):
                    outproj_chunk(po_pool, 3, ct, split_dma=(ct >= 4))
                for hh in range(2):
                    outproj_half(po_pool, 3, 6, hh)
                for hh in range(2):
                    outproj_half(po_pool, 3, 7, hh)

    nc.compile()
    return nc


def _get_nc():
    if "nc" not in _CACHE:
        _CACHE["nc"] = _build()
    return _CACHE["nc"]


def kernel(x, w_qkv, b_qkv, w_out, b_out):
    import ml_dtypes
    from concourse.bass_utils import run_bass_kernel_spmd

    bf = ml_dtypes.bfloat16
    x = np.asarray(x, dtype=np.float32)
    w_qkv = np.asarray(w_qkv, dtype=np.float32)
    b_qkv = np.asarray(b_qkv, dtype=np.float32)
    w_out = np.asarray(w_out, dtype=np.float32)
    b_out = np.asarray(b_out, dtype=np.float32)

    in_maps = []
    for core in range(8):
        b = core // 2
        hg = core % 2
        cs = hg * 512
        wqk = np.empty((C, 1024), dtype=bf)
        wqk[:, 0:512] = w_qkv[:, cs:cs + 512]
        wqk[:, 512:1024] = w_qkv[:, C + cs:C + cs + 512]
        bqk = np.empty((128, 8), dtype=np.float32)
        for j in range(4):
            bqk[:, j] = b_qkv[cs + j * 128: cs + (j + 1) * 128]
            bqk[:, 4 + j] = b_qkv[C + cs + j * 128: C + cs + (j + 1) * 128]
        in_maps.append({
            "xt": np.ascontiguousarray(x[b].T).astype(bf),
            "wqk": wqk,
            "wv": w_qkv[:, 2 * C + cs:2 * C + cs + 512].astype(bf),
            "wo": np.ascontiguousarray(w_out[cs:cs + 512, :]).astype(bf),
            "bqk": bqk,
        })

    _CACHE["in_maps"] = in_maps
    res = run_bass_kernel_spmd(_get_nc(), in_maps, core_ids=list(range(8)))

    # host epilogue: sum head-group partials, add b_out and the V-bias term
    b_eff = b_out + b_qkv[2 * C:3 * C] @ w_out
    out = np.empty((B, T, C), dtype=np.float32)
    for b in range(B):
        acc = (res.results[2 * b]["outT"].astype(np.float32)
               + res.results[2 * b + 1]["outT"].astype(np.float32))
        out[b] = acc.T + b_eff[None, :]
    return out


# revision 7
# speedup vs baseline: 1.0060x; 1.0040x over previous
"""Causal self-attention Bass/Tile kernel for 8 Trainium2 NeuronCores (v3).

Problem: B=4, T=2048, C=1024, H=16, D=64 (fp32 in/out).
  qkv = x @ w_qkv + b_qkv ; causal softmax attention ; y @ w_out + b_out

Sharding (8 cores): core i handles batch b = i//2 and head-group hg = i%2
(8 of the 16 heads). Host sums the two partial output projections per batch
and adds b_out + b_v @ w_out (the V-bias commutes through attention).

v3 = v2 + software-pipelined phases: the QKV projection for T-chunk t+1 is
emitted interleaved with attention for q-chunks that only need chunks <= t,
so the Act engine's softmax-exp (the attention-phase bottleneck) overlaps
the PE-bound projection work instead of following it.

Key measured-on-HW facts this design is built on:
  - bf16 matmul: 1 col/cycle; fp32r: 2; M and K do not affect cost.
  - Two K=64 quadrant matmuls (tile_position (0,0)/(64,0)) stream
    concurrently: a head-pair's score tiles cost ~224 ns per 2x[128x512].
  - A PSUM bank must only ever be written by one PE tile geometry within a
    pool scope (mixing quadrant and full tiles on a bank faults the device).
  - Act exp costs ~0.83 ns/elem + ~260 ns/instruction: score tiles are
    [128,1024] (2 banks, one head-pair) so one exp covers both heads, and
    diagonal tiles exp only the live [d:512] columns.
  - The causal mask is a bf16 lower-triangle multiply on the DVE (post-exp)
    on the single 128-wide diagonal block.
  - Softmax denominators ride along as 64 ones-columns in the PV stationary
    ([ones64 | v64] per head, M=128): the denominator lands replicated in
    PSUM rows 0:64, so normalization is reciprocal_approx_fast + one
    tensor_mul: no partition_broadcast, no extra matmuls.
  - Weights are SBUF-resident in bf16; out-projection chunks are interleaved
    into later attention blocks to fill Act-gated PE bubbles; output is bf16.
"""

import numpy as np

B, T, C = 4, 2048, 1024
H, D = 16, 64
HL = 8          # heads per core
HP = HL // 2    # head-pairs per core
KCH = C // 128  # 8 contraction chunks
TCH = T // 512  # 4 T chunks of 512
SCALE = 1.0 / 8.0  # 1/sqrt(D)

_CACHE = {}


def _build():
    import concourse.bass as bass  # noqa: F401
    import concourse.mybir as mybir
    import concourse.tile as tile
    from concourse import bacc

    f32 = mybir.dt.float32
    bf16 = mybir.dt.bfloat16
    Exp = mybir.ActivationFunctionType.Exp
    Ident = mybir.ActivationFunctionType.Identity

    nc = bacc.Bacc("TRN2", target_bir_lowering=False, debug=False, num_devices=8)

    xt_d = nc.dram_tensor("xt", [C, T], bf16, kind="ExternalInput")
    wqk_d = nc.dram_tensor("wqk", [C, 1024], bf16, kind="ExternalInput")
    wv_d = nc.dram_tensor("wv", [C, 512], bf16, kind="ExternalInput")
    wo_d = nc.dram_tensor("wo", [512, 1024], bf16, kind="ExternalInput")
    bqk_d = nc.dram_tensor("bqk", [128, 8], f32, kind="ExternalInput")
    out_d = nc.dram_tensor("outT", [1024, T], bf16, kind="ExternalOutput")

    with tile.TileContext(nc) as tc, nc.allow_low_precision(
        reason="bf16 matmul pipeline; rel error budget 2e-2"
    ):
        with (
            tc.tile_pool(name="wq", bufs=1) as w_pool,
            tc.tile_pool(name="qt", bufs=HP) as qt_pool,
            tc.tile_pool(name="kt", bufs=HP) as kt_pool,
            tc.tile_pool(name="v", bufs=16) as v_pool,
            tc.tile_pool(name="yt", bufs=HP) as yt_pool,
            tc.tile_pool(name="p", bufs=4) as p_pool,
            tc.tile_pool(name="misc", bufs=1) as misc_pool,
            tc.tile_pool(name="rcp", bufs=4) as rcp_pool,
            tc.tile_pool(name="xt", bufs=16) as xt_pool,
            tc.tile_pool(name="ostage", bufs=3) as ostage_pool,
            tc.tile_pool(name="ps_y", bufs=1, space="PSUM") as ps_y,
        ):
            # ---- resident weights ----
            wqk_sb = [w_pool.tile([128, 1024], bf16, tag=f"wqk{k}", name=f"wqk{k}")
                      for k in range(KCH)]
            wv_sb = [w_pool.tile([128, 512], bf16, tag=f"wv{k}", name=f"wv{k}")
                     for k in range(KCH)]
            wo_sb = [w_pool.tile([128, 1024], bf16, tag=f"wo{k}", name=f"wo{k}")
                     for k in range(4)]
            bqk_sb = misc_pool.tile([128, 8], f32, tag="bqk", name="bqk")
            nc.sync.dma_start(out=bqk_sb[:], in_=bqk_d[:])

            # lower-triangle (keep j >= p) bf16 mask, two copies side by side
            tri = misc_pool.tile([128, 256], bf16, tag="tri", name="tri")
            nc.vector.memset(tri[:], 1.0)
            for half in range(2):
                nc.gpsimd.affine_select(
                    out=tri[:, half * 128:(half + 1) * 128],
                    in_=tri[:, half * 128:(half + 1) * 128],
                    compare_op=mybir.AluOpType.is_ge,
                    fill=0.0, base=0, pattern=[[1, 128]], channel_multiplier=-1)

            qt = [qt_pool.tile([128, T], bf16, tag="qt", name="qt") for _ in range(HP)]
            kt = [kt_pool.tile([128, T], bf16, tag="kt", name="kt") for _ in range(HP)]
            v_sb = [v_pool.tile([128, 1024], bf16, tag="v", name="v") for _ in range(16)]
            yt = [yt_pool.tile([128, T], bf16, tag="yt", name="yt") for _ in range(HP)]

            xtc_all = {}

            def emit_xtc_dma(tch):
                ts = tch * 512
                xtc = [xt_pool.tile([128, 512], bf16, tag="xt", name="xt")
                       for _ in range(KCH)]
                xtc_all[tch] = xtc
                if tch == 0:
                    # critical path of the first matmul: xtc[0] + wqk[0]
                    nc.sync.dma_start(out=xtc[0][:], in_=xt_d[0:128, 0:512])
                    for qq in range(4):
                        nc.sync.dma_start(
                            out=wqk_sb[0][:, qq * 256:(qq + 1) * 256],
                            in_=wqk_d[0:128, qq * 256:(qq + 1) * 256])
                for k in range(KCH):
                    if tch == 0 and k == 0:
                        continue
                    nc.sync.dma_start(
                        out=xtc[k][:],
                        in_=xt_d[k * 128:(k + 1) * 128, ts:ts + 512])
                if tch == 0:
                    # weight DMAs issue from the (early-idle) Act queue in
                    # parallel with the sync queue's xtc stream
                    for k in range(1, KCH):
                        for hh in range(2):
                            nc.scalar.dma_start(
                                out=wqk_sb[k][:, hh * 512:(hh + 1) * 512],
                                in_=wqk_d[k * 128:(k + 1) * 128,
                                          hh * 512:(hh + 1) * 512])
                    for k in range(KCH):
                        nc.sync.dma_start(
                            out=wv_sb[k][:], in_=wv_d[k * 128:(k + 1) * 128, :])
                if tch == 1:
                    for k in range(4):
                        nc.sync.dma_start(
                            out=wo_sb[k][:], in_=wo_d[k * 128:(k + 1) * 128, :])

            def qk_chain(pj_pool, tch, ct):
                ts = tch * 512
                xtc = xtc_all[tch]
                pj = pj_pool.tile([128, 512], f32, tag="pj", name="pj")
                for k in range(KCH):
                    nc.tensor.matmul(
                        pj[:], wqk_sb[k][:, ct * 128:(ct + 1) * 128], xtc[k][:],
                        start=(k == 0), stop=(k == KCH - 1))
                dst = qt[ct] if ct < HP else kt[ct - HP]
                if ct % 2 == 0:
                    nc.scalar.activation(dst[:, ts:ts + 512], pj[:], Ident,
                                         bias=bqk_sb[:, ct:ct + 1])
                else:
                    nc.vector.tensor_scalar_add(dst[:, ts:ts + 512], pj[:],
                                                bqk_sb[:, ct:ct + 1])

            def v_chain(pj_pool, tch, tl):
                xtc = xtc_all[tch]
                pj = pj_pool.tile([128, 512], f32, tag="pj", name="pj")
                for k in range(KCH):
                    nc.tensor.matmul(
                        pj[:], xtc[k][:, tl * 128:(tl + 1) * 128], wv_sb[k][:],
                        start=(k == 0), stop=(k == KCH - 1))
                tt = tch * 4 + tl
                vt = v_sb[tt]
                v_view = vt[:].rearrange("p (h c) -> p h c", c=128)
                srcv = pj[:].rearrange("p (h c) -> p h c", c=64)
                if tl % 2 == 0:
                    nc.scalar.activation(v_view[:, :, 64:128], srcv, Ident)
                else:
                    nc.vector.tensor_copy(v_view[:, :, 64:128], srcv)
                nc.gpsimd.memset(v_view[:, :, 0:64], 1.0)

            def attn_block(s_pool, qc, hp):
                """Generator: one (qc, hp) attention block, yielding after
                each kti unit so the driver can interleave other PE work."""
                qs = qc * 512
                ya = ps_y.tile([128, 512], f32, tag="ya", name="ya")
                yb = ps_y.tile([128, 512], f32, tag="yb", name="yb")
                emit = [4 * qc + j for j in range(4)] + list(range(4 * qc))

                def flush_pv(kti, p_t, d, ia, ib):
                    w0 = 0 if qc == 0 else max(d, 0)
                    ha, hb = 2 * hp, 2 * hp + 1
                    nc.tensor.matmul(
                        ya[:, w0:512],
                        v_sb[kti][:, ha * 128:(ha + 1) * 128],
                        p_t[:, w0:512], start=ia, stop=ib)
                    nc.tensor.matmul(
                        yb[:, w0:512],
                        v_sb[kti][:, hb * 128:(hb + 1) * 128],
                        p_t[:, 512 + w0:1024], start=ia, stop=ib)

                pend = []
                for kti in emit:
                    ks = kti * 128
                    d = ks - qs
                    w0 = max(d, 0)
                    s_t = s_pool.tile([128, 1024], f32, tag="s", name="s")
                    nc.tensor.matmul(
                        s_t[:, w0:512],
                        kt[hp][0:64, ks:ks + 128],
                        qt[hp][0:64, qs + w0:qs + 512],
                        start=True, stop=True, tile_position=(0, 0))
                    nc.tensor.matmul(
                        s_t[:, 512 + w0:1024],
                        kt[hp][64:128, ks:ks + 128],
                        qt[hp][64:128, qs + w0:qs + 512],
                        start=True, stop=True, tile_position=(64, 0))
                    p_t = p_pool.tile([128, 1024], bf16, tag="p", name="p")
                    sv = s_t[:].rearrange("p (b c) -> p b c", c=512)
                    pv = p_t[:].rearrange("p (b c) -> p b c", c=512)
                    nc.scalar.activation(pv[:, :, w0:512], sv[:, :, w0:512],
                                         Exp, scale=SCALE)
                    if d >= 0:
                        nc.vector.tensor_mul(
                            pv[:, :, d:d + 128], pv[:, :, d:d + 128],
                            tri[:].rearrange("p (b c) -> p b c", c=128))
                        if qc == 0 and w0 > 0:
                            nc.gpsimd.memset(pv[:, :, 0:w0], 0.0)
                    pend.append((kti, p_t, d))
                    if len(pend) > 1:
                        k0, p0, d0 = pend.pop(0)
                        flush_pv(k0, p0, d0, k0 == emit[0], False)
                    yield
                k0, p0, d0 = pend.pop(0)
                flush_pv(k0, p0, d0, k0 == emit[0], True)
                # normalize: y rows 64:128 / replicated denominator rows 0:64
                for off, yy in ((0, ya), (64, yb)):
                    rcp = rcp_pool.tile([64, 512], f32, tag="rcp", name="rcp")
                    nc.vector.reciprocal_approx_fast(out=rcp[:], in_=yy[0:64, :])
                    nc.vector.tensor_mul(
                        yt[hp][off:off + 64, qs:qs + 512],
                        yy[64:128, :], rcp[:])
                yield

            def outproj_chunk(po_pool, qc_, ct, split_dma=False):
                qs_ = qc_ * 512
                po = po_pool.tile([128, 512], f32, tag="po", name="po")
                for k in range(4):
                    nc.tensor.matmul(
                        po[:], wo_sb[k][:, ct * 128:(ct + 1) * 128],
                        yt[k][:, qs_:qs_ + 512],
                        start=(k == 0), stop=(k == 3))
                st = ostage_pool.tile([128, 512], bf16, tag="ost", name="ost")
                nc.vector.tensor_copy(st[:], po[:])
                if split_dma:
                    # tail chunks: halve per-queue transfer time
                    for hh in range(2):
                        nc.sync.dma_start(
                            out=out_d[ct * 128:(ct + 1) * 128,
                                      qs_ + hh * 256:qs_ + (hh + 1) * 256],
                            in_=st[:, hh * 256:(hh + 1) * 256])
                else:
                    nc.sync.dma_start(
                        out=out_d[ct * 128:(ct + 1) * 128, qs_:qs_ + 512],
                        in_=st[:])

            def outproj_half(po_pool, qc_, ct, hh):
                qs_ = qc_ * 512 + hh * 256
                pof = po_pool.tile([128, 512], f32, tag="po", name="poh")
                po = pof[:, 0:256]
                for k in range(4):
                    nc.tensor.matmul(
                        po, wo_sb[k][:, ct * 128:(ct + 1) * 128],
                        yt[k][:, qs_:qs_ + 256],
                        start=(k == 0), stop=(k == 3))
                st = ostage_pool.tile([128, 256], bf16, tag="osth", name="osth")
                nc.vector.tensor_copy(st[:], po)
                nc.sync.dma_start(
                    out=out_d[ct * 128:(ct + 1) * 128, qs_:qs_ + 256],
                    in_=st[:])

            def drain(g):
                for _ in g:
                    pass

            # ====== scope 0: tch0 projection, k-outer across 6 banks ======
            # (k-inner chains would stall on weight-DMA arrival order here;
            # k-outer matches the DMA issue order so PE streams immediately)
            emit_xtc_dma(0)
            emit_xtc_dma(1)
            with tc.tile_pool(name="pj0", bufs=6, space="PSUM") as pj0:
                xtc = xtc_all[0]
                pja = [pj0.tile([128, 512], f32, tag="pj", name="pj")
                       for _ in range(6)]
                for k in range(KCH):
                    for ct in range(6):
                        nc.tensor.matmul(
                            pja[ct][:], wqk_sb[k][:, ct * 128:(ct + 1) * 128],
                            xtc[k][:], start=(k == 0), stop=(k == KCH - 1))
                for ct in range(6):
                    dst = qt[ct] if ct < HP else kt[ct - HP]
                    nc.vector.tensor_scalar_add(dst[:, 0:512], pja[ct][:],
                                                bqk_sb[:, ct:ct + 1])
                pjb = [pj0.tile([128, 512], f32, tag="pj", name="pj")
                       for _ in range(2)]
                for k in range(KCH):
                    for i, ct in enumerate((6, 7)):
                        nc.tensor.matmul(
                            pjb[i][:], wqk_sb[k][:, ct * 128:(ct + 1) * 128],
                            xtc[k][:], start=(k == 0), stop=(k == KCH - 1))
                for i, ct in enumerate((6, 7)):
                    dst = kt[ct - HP]
                    nc.vector.tensor_scalar_add(dst[:, 0:512], pjb[i][:],
                                                bqk_sb[:, ct:ct + 1])
                pjv = [pj0.tile([128, 512], f32, tag="pj", name="pj")
                       for _ in range(4)]
                for k in range(KCH):
                    for tl in range(4):
                        nc.tensor.matmul(
                            pjv[tl][:], xtc[k][:, tl * 128:(tl + 1) * 128],
                            wv_sb[k][:], start=(k == 0), stop=(k == KCH - 1))
                for tl in range(4):
                    vt = v_sb[tl]
                    v_view = vt[:].rearrange("p (h c) -> p h c", c=128)
                    srcv = pjv[tl][:].rearrange("p (h c) -> p h c", c=64)
                    if tl % 2 == 0:
                        nc.scalar.activation(v_view[:, :, 64:128], srcv, Ident)
                    else:
                        nc.vector.tensor_copy(v_view[:, :, 64:128], srcv)
                    nc.gpsimd.memset(v_view[:, :, 0:64], 1.0)

            # ================= scope 1: tch1-3 chains + qc0..qc2(hp0,1) =====
            with (
                tc.tile_pool(name="pj", bufs=2, space="PSUM") as pj_pool,
                tc.tile_pool(name="s1", bufs=2, space="PSUM") as s1_pool,
            ):

                # attention unit streams, eligible per completed tch
                streams = []
                for qc, hps in ((0, range(HP)), (1, range(HP)), (2, (0, 1))):
                    for hp in hps:
                        streams.append((qc, attn_block(s1_pool, qc, hp)))
                si = 0          # index into streams
                cur = None

                def next_unit(max_qc):
                    nonlocal si, cur
                    while si < len(streams):
                        qc, g = streams[si]
                        if qc > max_qc:
                            return False
                        try:
                            next(g)
                            return True
                        except StopIteration:
                            si += 1
                    return False

                # tch1 chains <-> qc0 units
                emit_xtc_dma(2)
                for ci, ct in enumerate(range(8)):
                    qk_chain(pj_pool, 1, ct)
                    next_unit(0)
                    if ci % 2 == 0:
                        next_unit(0)
                for tl in range(4):
                    v_chain(pj_pool, 1, tl)
                    next_unit(0)
                # tch2 chains <-> qc1 units
                emit_xtc_dma(3)
                for ct in range(8):
                    qk_chain(pj_pool, 2, ct)
                    next_unit(1)
                    next_unit(1)
                for tl in range(4):
                    v_chain(pj_pool, 2, tl)
                    next_unit(1)
                    next_unit(1)
                # tch3 chains <-> qc1 rest + qc2(hp0,1) units
                for ct in range(8):
                    qk_chain(pj_pool, 3, ct)
                    next_unit(2)
                    next_unit(2)
                    next_unit(2)
                for tl in range(4):
                    v_chain(pj_pool, 3, tl)
                    next_unit(2)
                    next_unit(2)
                    next_unit(2)
                # finish remaining scope-1 attention
                while next_unit(2):
                    pass

            # ===== scope 2: qc2(hp2,3) + qc3 + all output projection ========
            with (
                tc.tile_pool(name="s2", bufs=2, space="PSUM") as s2_pool,
                tc.tile_pool(name="ps_o", bufs=2, space="PSUM") as po_pool,
            ):
                def drive(gen, chunks):
                    # interleave filler out-proj chunks INSIDE the kti loop
                    # so PE filler tracks the Act deficit as it accrues
                    n = 0
                    ci = 0
                    for _ in gen:
                        n += 1
                        if ci < len(chunks) and n % 3 == 0:
                            qq, ct = chunks[ci]
                            ci += 1
                            outproj_chunk(po_pool, qq, ct)
                    while ci < len(chunks):
                        qq, ct = chunks[ci]
                        ci += 1
                        outproj_chunk(po_pool, qq, ct)

                # qc2 hp2/hp3 with qc0's out-proj interleaved
                for i, hp in enumerate((2, 3)):
                    drive(attn_block(s2_pool, 2, hp),
                          [(0, ct) for ct in range(4 * i, 4 * i + 4)])
                # qc3 with qc1/qc2 out-proj interleaved
                for hp in range(HP):
                    drive(attn_block(s2_pool, 3, hp),
                          [(1, 2 * hp), (1, 2 * hp + 1),
                           (2, 2 * hp), (2, 2 * hp + 1)])
                # last q-chunk: first 6 full, last 2 as half-width pieces so
                # the serial matmul->copy->DMA tail is shorter
                for ct in range(6):
                    outproj_chunk(po_pool, 3, ct, split_dma=(ct >= 4))
                for hh in range(2):
                    outproj_half(po_pool, 3, 6, hh)
                for hh in range(2):
                    outproj_half(po_pool, 3, 7, hh)

    nc.compile()
    return nc


def _get_nc():
    if "nc" not in _CACHE:
        _CACHE["nc"] = _build()
    return _CACHE["nc"]


def kernel(x, w_qkv, b_qkv, w_out, b_out):
    import ml_dtypes
    from concourse.bass_utils import run_bass_kernel_spmd

    bf = ml_dtypes.bfloat16
    x = np.asarray(x, dtype=np.float32)
    w_qkv = np.asarray(w_qkv, dtype=np.float32)
    b_qkv = np.asarray(b_qkv, dtype=np.float32)
    w_out = np.asarray(w_out, dtype=np.float32)
    b_out = np.asarray(b_out, dtype=np.float32)

    in_maps = []
    for core in range(8):
        b = core // 2
        hg = core % 2
        cs = hg * 512
        wqk = np.empty((C, 1024), dtype=bf)
        wqk[:, 0:512] = w_qkv[:, cs:cs + 512]
        wqk[:, 512:1024] = w_qkv[:, C + cs:C + cs + 512]
        bqk = np.empty((128, 8), dtype=np.float32)
        for j in range(4):
            bqk[:, j] = b_qkv[cs + j * 128: cs + (j + 1) * 128]
            bqk[:, 4 + j] = b_qkv[C + cs + j * 128: C + cs + (j + 1) * 128]
        in_maps.append({
            "xt": np.ascontiguousarray(x[b].T).astype(bf),
            "wqk": wqk,
            "wv": w_qkv[:, 2 * C + cs:2 * C + cs + 512].astype(bf),
            "wo": np.ascontiguousarray(w_out[cs:cs + 512, :]).astype(bf),
            "bqk": bqk,
        })

    _CACHE["in_maps"] = in_maps
    res = run_bass_kernel_spmd(_get_nc(), in_maps, core_ids=list(range(8)))

    # host epilogue: sum head-group partials, add b_out and the V-bias term
    b_eff = b_out + b_qkv[2 * C:3 * C] @ w_out
    out = np.empty((B, T, C), dtype=np.float32)
    for b in range(B):
        acc = (res.results[2 * b]["outT"].astype(np.float32)
               + res.results[2 * b + 1]["outT"].astype(np.float32))
        out[b] = acc.T + b_eff[None, :]
    return out


# revision 8
# speedup vs baseline: 1.0099x; 1.0038x over previous
"""Causal self-attention Bass/Tile kernel for 8 Trainium2 NeuronCores (v3).

Problem: B=4, T=2048, C=1024, H=16, D=64 (fp32 in/out).
  qkv = x @ w_qkv + b_qkv ; causal softmax attention ; y @ w_out + b_out

Sharding (8 cores): core i handles batch b = i//2 and head-group hg = i%2
(8 of the 16 heads). Host sums the two partial output projections per batch
and adds b_out + b_v @ w_out (the V-bias commutes through attention).

v3 = v2 + software-pipelined phases: the QKV projection for T-chunk t+1 is
emitted interleaved with attention for q-chunks that only need chunks <= t,
so the Act engine's softmax-exp (the attention-phase bottleneck) overlaps
the PE-bound projection work instead of following it.

Key measured-on-HW facts this design is built on:
  - bf16 matmul: 1 col/cycle; fp32r: 2; M and K do not affect cost.
  - Two K=64 quadrant matmuls (tile_position (0,0)/(64,0)) stream
    concurrently: a head-pair's score tiles cost ~224 ns per 2x[128x512].
  - A PSUM bank must only ever be written by one PE tile geometry within a
    pool scope (mixing quadrant and full tiles on a bank faults the device).
  - Act exp costs ~0.83 ns/elem + ~260 ns/instruction: score tiles are
    [128,1024] (2 banks, one head-pair) so one exp covers both heads, and
    diagonal tiles exp only the live [d:512] columns.
  - The causal mask is a bf16 lower-triangle multiply on the DVE (post-exp)
    on the single 128-wide diagonal block.
  - Softmax denominators ride along as 64 ones-columns in the PV stationary
    ([ones64 | v64] per head, M=128): the denominator lands replicated in
    PSUM rows 0:64, so normalization is reciprocal_approx_fast + one
    tensor_mul: no partition_broadcast, no extra matmuls.
  - Weights are SBUF-resident in bf16; out-projection chunks are interleaved
    into later attention blocks to fill Act-gated PE bubbles; output is bf16.
"""

import numpy as np

B, T, C = 4, 2048, 1024
H, D = 16, 64
HL = 8          # heads per core
HP = HL // 2    # head-pairs per core
KCH = C // 128  # 8 contraction chunks
TCH = T // 512  # 4 T chunks of 512
SCALE = 1.0 / 8.0  # 1/sqrt(D)

_CACHE = {}


def _build():
    import concourse.bass as bass  # noqa: F401
    import concourse.mybir as mybir
    import concourse.tile as tile
    from concourse import bacc

    f32 = mybir.dt.float32
    bf16 = mybir.dt.bfloat16
    Exp = mybir.ActivationFunctionType.Exp
    Ident = mybir.ActivationFunctionType.Identity

    nc = bacc.Bacc("TRN2", target_bir_lowering=False, debug=False, num_devices=8)

    xt_d = nc.dram_tensor("xt", [C, T], bf16, kind="ExternalInput")
    wqk_d = nc.dram_tensor("wqk", [C, 1024], bf16, kind="ExternalInput")
    wv_d = nc.dram_tensor("wv", [C, 512], bf16, kind="ExternalInput")
    wo_d = nc.dram_tensor("wo", [512, 1024], bf16, kind="ExternalInput")
    bqk_d = nc.dram_tensor("bqk", [128, 8], f32, kind="ExternalInput")
    out_d = nc.dram_tensor("outT", [1024, T], bf16, kind="ExternalOutput")

    with tile.TileContext(nc) as tc, nc.allow_low_precision(
        reason="bf16 matmul pipeline; rel error budget 2e-2"
    ):
        with (
            tc.tile_pool(name="wq", bufs=1) as w_pool,
            tc.tile_pool(name="qt", bufs=HP) as qt_pool,
            tc.tile_pool(name="kt", bufs=HP) as kt_pool,
            tc.tile_pool(name="v", bufs=16) as v_pool,
            tc.tile_pool(name="yt", bufs=HP) as yt_pool,
            tc.tile_pool(name="p", bufs=6) as p_pool,
            tc.tile_pool(name="misc", bufs=1) as misc_pool,
            tc.tile_pool(name="rcp", bufs=4) as rcp_pool,
            tc.tile_pool(name="xt", bufs=16) as xt_pool,
            tc.tile_pool(name="ostage", bufs=6) as ostage_pool,
            tc.tile_pool(name="ps_y", bufs=1, space="PSUM") as ps_y,
        ):
            # ---- resident weights ----
            wqk_sb = [w_pool.tile([128, 1024], bf16, tag=f"wqk{k}", name=f"wqk{k}")
                      for k in range(KCH)]
            wv_sb = [w_pool.tile([128, 512], bf16, tag=f"wv{k}", name=f"wv{k}")
                     for k in range(KCH)]
            wo_sb = [w_pool.tile([128, 1024], bf16, tag=f"wo{k}", name=f"wo{k}")
                     for k in range(4)]
            bqk_sb = misc_pool.tile([128, 8], f32, tag="bqk", name="bqk")
            nc.sync.dma_start(out=bqk_sb[:], in_=bqk_d[:])

            # lower-triangle (keep j >= p) bf16 mask, two copies side by side
            tri = misc_pool.tile([128, 256], bf16, tag="tri", name="tri")
            nc.vector.memset(tri[:], 1.0)
            for half in range(2):
                nc.gpsimd.affine_select(
                    out=tri[:, half * 128:(half + 1) * 128],
                    in_=tri[:, half * 128:(half + 1) * 128],
                    compare_op=mybir.AluOpType.is_ge,
                    fill=0.0, base=0, pattern=[[1, 128]], channel_multiplier=-1)

            qt = [qt_pool.tile([128, T], bf16, tag="qt", name="qt") for _ in range(HP)]
            kt = [kt_pool.tile([128, T], bf16, tag="kt", name="kt") for _ in range(HP)]
            v_sb = [v_pool.tile([128, 1024], bf16, tag="v", name="v") for _ in range(16)]
            yt = [yt_pool.tile([128, T], bf16, tag="yt", name="yt") for _ in range(HP)]

            xtc_all = {}

            def emit_xtc_dma(tch):
                ts = tch * 512
                xtc = [xt_pool.tile([128, 512], bf16, tag="xt", name="xt")
                       for _ in range(KCH)]
                xtc_all[tch] = xtc
                if tch == 0:
                    # critical path of the first matmul: xtc[0] + wqk[0]
                    nc.sync.dma_start(out=xtc[0][:], in_=xt_d[0:128, 0:512])
                    for qq in range(4):
                        nc.sync.dma_start(
                            out=wqk_sb[0][:, qq * 256:(qq + 1) * 256],
                            in_=wqk_d[0:128, qq * 256:(qq + 1) * 256])
                for k in range(KCH):
                    if tch == 0 and k == 0:
                        continue
                    nc.sync.dma_start(
                        out=xtc[k][:],
                        in_=xt_d[k * 128:(k + 1) * 128, ts:ts + 512])
                if tch == 0:
                    # weight DMAs issue from the (early-idle) Act queue in
                    # parallel with the sync queue's xtc stream
                    for k in range(1, KCH):
                        for hh in range(2):
                            nc.scalar.dma_start(
                                out=wqk_sb[k][:, hh * 512:(hh + 1) * 512],
                                in_=wqk_d[k * 128:(k + 1) * 128,
                                          hh * 512:(hh + 1) * 512])
                    for k in range(KCH):
                        nc.sync.dma_start(
                            out=wv_sb[k][:], in_=wv_d[k * 128:(k + 1) * 128, :])
                if tch == 1:
                    for k in range(4):
                        nc.sync.dma_start(
                            out=wo_sb[k][:], in_=wo_d[k * 128:(k + 1) * 128, :])

            def qk_chain(pj_pool, tch, ct):
                ts = tch * 512
                xtc = xtc_all[tch]
                pj = pj_pool.tile([128, 512], f32, tag="pj", name="pj")
                for k in range(KCH):
                    nc.tensor.matmul(
                        pj[:], wqk_sb[k][:, ct * 128:(ct + 1) * 128], xtc[k][:],
                        start=(k == 0), stop=(k == KCH - 1))
                dst = qt[ct] if ct < HP else kt[ct - HP]
                if ct % 2 == 0:
                    nc.scalar.activation(dst[:, ts:ts + 512], pj[:], Ident,
                                         bias=bqk_sb[:, ct:ct + 1])
                else:
                    nc.vector.tensor_scalar_add(dst[:, ts:ts + 512], pj[:],
                                                bqk_sb[:, ct:ct + 1])

            def v_chain(pj_pool, tch, tl):
                xtc = xtc_all[tch]
                pj = pj_pool.tile([128, 512], f32, tag="pj", name="pj")
                for k in range(KCH):
                    nc.tensor.matmul(
                        pj[:], xtc[k][:, tl * 128:(tl + 1) * 128], wv_sb[k][:],
                        start=(k == 0), stop=(k == KCH - 1))
                tt = tch * 4 + tl
                vt = v_sb[tt]
                v_view = vt[:].rearrange("p (h c) -> p h c", c=128)
                srcv = pj[:].rearrange("p (h c) -> p h c", c=64)
                if tl % 2 == 0:
                    nc.scalar.activation(v_view[:, :, 64:128], srcv, Ident)
                else:
                    nc.vector.tensor_copy(v_view[:, :, 64:128], srcv)
                nc.gpsimd.memset(v_view[:, :, 0:64], 1.0)

            def attn_block(s_pool, qc, hp):
                """Generator: one (qc, hp) attention block, yielding after
                each kti unit so the driver can interleave other PE work."""
                qs = qc * 512
                ya = ps_y.tile([128, 512], f32, tag="ya", name="ya")
                yb = ps_y.tile([128, 512], f32, tag="yb", name="yb")
                emit = [4 * qc + j for j in range(4)] + list(range(4 * qc))

                def flush_pv(kti, p_t, d, ia, ib):
                    w0 = 0 if qc == 0 else max(d, 0)
                    ha, hb = 2 * hp, 2 * hp + 1
                    nc.tensor.matmul(
                        ya[:, w0:512],
                        v_sb[kti][:, ha * 128:(ha + 1) * 128],
                        p_t[:, w0:512], start=ia, stop=ib)
                    nc.tensor.matmul(
                        yb[:, w0:512],
                        v_sb[kti][:, hb * 128:(hb + 1) * 128],
                        p_t[:, 512 + w0:1024], start=ia, stop=ib)

                pend = []
                for kti in emit:
                    ks = kti * 128
                    d = ks - qs
                    w0 = max(d, 0)
                    s_t = s_pool.tile([128, 1024], f32, tag="s", name="s")
                    nc.tensor.matmul(
                        s_t[:, w0:512],
                        kt[hp][0:64, ks:ks + 128],
                        qt[hp][0:64, qs + w0:qs + 512],
                        start=True, stop=True, tile_position=(0, 0))
                    nc.tensor.matmul(
                        s_t[:, 512 + w0:1024],
                        kt[hp][64:128, ks:ks + 128],
                        qt[hp][64:128, qs + w0:qs + 512],
                        start=True, stop=True, tile_position=(64, 0))
                    p_t = p_pool.tile([128, 1024], bf16, tag="p", name="p")
                    sv = s_t[:].rearrange("p (b c) -> p b c", c=512)
                    pv = p_t[:].rearrange("p (b c) -> p b c", c=512)
                    nc.scalar.activation(pv[:, :, w0:512], sv[:, :, w0:512],
                                         Exp, scale=SCALE)
                    if d >= 0:
                        nc.vector.tensor_mul(
                            pv[:, :, d:d + 128], pv[:, :, d:d + 128],
                            tri[:].rearrange("p (b c) -> p b c", c=128))
                        if qc == 0 and w0 > 0:
                            nc.gpsimd.memset(pv[:, :, 0:w0], 0.0)
                    pend.append((kti, p_t, d))
                    if len(pend) > 1:
                        k0, p0, d0 = pend.pop(0)
                        flush_pv(k0, p0, d0, k0 == emit[0], False)
                    yield
                k0, p0, d0 = pend.pop(0)
                flush_pv(k0, p0, d0, k0 == emit[0], True)
                # normalize: y rows 64:128 / replicated denominator rows 0:64
                for off, yy in ((0, ya), (64, yb)):
                    rcp = rcp_pool.tile([64, 512], f32, tag="rcp", name="rcp")
                    nc.vector.reciprocal_approx_fast(out=rcp[:], in_=yy[0:64, :])
                    nc.vector.tensor_mul(
                        yt[hp][off:off + 64, qs:qs + 512],
                        yy[64:128, :], rcp[:])
                yield

            def outproj_chunk(po_pool, qc_, ct, split_dma=False):
                qs_ = qc_ * 512
                po = po_pool.tile([128, 512], f32, tag="po", name="po")
                for k in range(4):
                    nc.tensor.matmul(
                        po[:], wo_sb[k][:, ct * 128:(ct + 1) * 128],
                        yt[k][:, qs_:qs_ + 512],
                        start=(k == 0), stop=(k == 3))
                st = ostage_pool.tile([128, 512], bf16, tag="ost", name="ost")
                nc.vector.tensor_copy(st[:], po[:])
                if split_dma:
                    # tail chunks: halve per-queue transfer time
                    for hh in range(2):
                        nc.sync.dma_start(
                            out=out_d[ct * 128:(ct + 1) * 128,
                                      qs_ + hh * 256:qs_ + (hh + 1) * 256],
                            in_=st[:, hh * 256:(hh + 1) * 256])
                else:
                    nc.sync.dma_start(
                        out=out_d[ct * 128:(ct + 1) * 128, qs_:qs_ + 512],
                        in_=st[:])

            def outproj_half(po_pool, qc_, ct, hh):
                qs_ = qc_ * 512 + hh * 256
                pof = po_pool.tile([128, 512], f32, tag="po", name="poh")
                po = pof[:, 0:256]
                for k in range(4):
                    nc.tensor.matmul(
                        po, wo_sb[k][:, ct * 128:(ct + 1) * 128],
                        yt[k][:, qs_:qs_ + 256],
                        start=(k == 0), stop=(k == 3))
                st = ostage_pool.tile([128, 256], bf16, tag="osth", name="osth")
                nc.vector.tensor_copy(st[:], po)
                nc.sync.dma_start(
                    out=out_d[ct * 128:(ct + 1) * 128, qs_:qs_ + 256],
                    in_=st[:])

            def drain(g):
                for _ in g:
                    pass

            # ====== scope 0: tch0 projection, k-outer across 6 banks ======
            # (k-inner chains would stall on weight-DMA arrival order here;
            # k-outer matches the DMA issue order so PE streams immediately)
            emit_xtc_dma(0)
            emit_xtc_dma(1)
            with tc.tile_pool(name="pj0", bufs=6, space="PSUM") as pj0:
                xtc = xtc_all[0]
                pja = [pj0.tile([128, 512], f32, tag="pj", name="pj")
                       for _ in range(6)]
                for k in range(KCH):
                    for ct in range(6):
                        nc.tensor.matmul(
                            pja[ct][:], wqk_sb[k][:, ct * 128:(ct + 1) * 128],
                            xtc[k][:], start=(k == 0), stop=(k == KCH - 1))
                for ct in range(6):
                    dst = qt[ct] if ct < HP else kt[ct - HP]
                    nc.vector.tensor_scalar_add(dst[:, 0:512], pja[ct][:],
                                                bqk_sb[:, ct:ct + 1])
                pjb = [pj0.tile([128, 512], f32, tag="pj", name="pj")
                       for _ in range(2)]
                for k in range(KCH):
                    for i, ct in enumerate((6, 7)):
                        nc.tensor.matmul(
                            pjb[i][:], wqk_sb[k][:, ct * 128:(ct + 1) * 128],
                            xtc[k][:], start=(k == 0), stop=(k == KCH - 1))
                for i, ct in enumerate((6, 7)):
                    dst = kt[ct - HP]
                    nc.vector.tensor_scalar_add(dst[:, 0:512], pjb[i][:],
                                                bqk_sb[:, ct:ct + 1])
                pjv = [pj0.tile([128, 512], f32, tag="pj", name="pj")
                       for _ in range(4)]
                for k in range(KCH):
                    for tl in range(4):
                        nc.tensor.matmul(
                            pjv[tl][:], xtc[k][:, tl * 128:(tl + 1) * 128],
                            wv_sb[k][:], start=(k == 0), stop=(k == KCH - 1))
                for tl in range(4):
                    vt = v_sb[tl]
                    v_view = vt[:].rearrange("p (h c) -> p h c", c=128)
                    srcv = pjv[tl][:].rearrange("p (h c) -> p h c", c=64)
                    if tl % 2 == 0:
                        nc.scalar.activation(v_view[:, :, 64:128], srcv, Ident)
                    else:
                        nc.vector.tensor_copy(v_view[:, :, 64:128], srcv)
                    nc.gpsimd.memset(v_view[:, :, 0:64], 1.0)

            # ================= scope 1: tch1-3 chains + qc0..qc2(hp0,1) =====
            with (
                tc.tile_pool(name="pj", bufs=2, space="PSUM") as pj_pool,
                tc.tile_pool(name="s1", bufs=2, space="PSUM") as s1_pool,
            ):

                # attention unit streams, eligible per completed tch
                streams = []
                for qc, hps in ((0, range(HP)), (1, range(HP)), (2, (0, 1))):
                    for hp in hps:
                        streams.append((qc, attn_block(s1_pool, qc, hp)))
                si = 0          # index into streams
                cur = None

                def next_unit(max_qc):
                    nonlocal si, cur
                    while si < len(streams):
                        qc, g = streams[si]
                        if qc > max_qc:
                            return False
                        try:
                            next(g)
                            return True
                        except StopIteration:
                            si += 1
                    return False

                # tch1 chains <-> qc0 units
                emit_xtc_dma(2)
                for ci, ct in enumerate(range(8)):
                    qk_chain(pj_pool, 1, ct)
                    next_unit(0)
                    if ci % 2 == 0:
                        next_unit(0)
                for tl in range(4):
                    v_chain(pj_pool, 1, tl)
                    next_unit(0)
                # tch2 chains <-> qc1 units
                emit_xtc_dma(3)
                for ct in range(8):
                    qk_chain(pj_pool, 2, ct)
                    next_unit(1)
                    next_unit(1)
                for tl in range(4):
                    v_chain(pj_pool, 2, tl)
                    next_unit(1)
                    next_unit(1)
                # tch3 chains <-> qc1 rest + qc2(hp0,1) units
                for ct in range(8):
                    qk_chain(pj_pool, 3, ct)
                    next_unit(2)
                    next_unit(2)
                    next_unit(2)
                for tl in range(4):
                    v_chain(pj_pool, 3, tl)
                    next_unit(2)
                    next_unit(2)
                    next_unit(2)
                # finish remaining scope-1 attention
                while next_unit(2):
                    pass

            # ===== scope 2: qc2(hp2,3) + qc3 + all output projection ========
            with (
                tc.tile_pool(name="s2", bufs=2, space="PSUM") as s2_pool,
                tc.tile_pool(name="ps_o", bufs=2, space="PSUM") as po_pool,
            ):
                def drive(gen, chunks):
                    # interleave filler out-proj chunks INSIDE the kti loop
                    # so PE filler tracks the Act deficit as it accrues
                    n = 0
                    ci = 0
                    for _ in gen:
                        n += 1
                        if ci < len(chunks) and n % 3 == 0:
                            qq, ct = chunks[ci]
                            ci += 1
                            outproj_chunk(po_pool, qq, ct)
                    while ci < len(chunks):
                        qq, ct = chunks[ci]
                        ci += 1
                        outproj_chunk(po_pool, qq, ct)

                # qc2 hp2/hp3 with qc0's out-proj interleaved
                for i, hp in enumerate((2, 3)):
                    drive(attn_block(s2_pool, 2, hp),
                          [(0, ct) for ct in range(4 * i, 4 * i + 4)])
                # qc3 with qc1/qc2 out-proj interleaved
                for hp in range(HP):
                    drive(attn_block(s2_pool, 3, hp),
                          [(1, 2 * hp), (1, 2 * hp + 1),
                           (2, 2 * hp), (2, 2 * hp + 1)])
                # last q-chunk: first 6 full, last 2 as half-width pieces so
                # the serial matmul->copy->DMA tail is shorter
                for ct in range(6):
                    outproj_chunk(po_pool, 3, ct, split_dma=(ct >= 4))
                for hh in range(2):
                    outproj_half(po_pool, 3, 6, hh)
                for hh in range(2):
                    outproj_half(po_pool, 3, 7, hh)

    nc.compile()
    return nc


def _get_nc():
    if "nc" not in _CACHE:
        _CACHE["nc"] = _build()
    return _CACHE["nc"]


def kernel(x, w_qkv, b_qkv, w_out, b_out):
    import ml_dtypes
    from concourse.bass_utils import run_bass_kernel_spmd

    bf = ml_dtypes.bfloat16
    x = np.asarray(x, dtype=np.float32)
    w_qkv = np.asarray(w_qkv, dtype=np.float32)
    b_qkv = np.asarray(b_qkv, dtype=np.float32)
    w_out = np.asarray(w_out, dtype=np.float32)
    b_out = np.asarray(b_out, dtype=np.float32)

    in_maps = []
    for core in range(8):
        b = core // 2
        hg = core % 2
        cs = hg * 512
        wqk = np.empty((C, 1024), dtype=bf)
        wqk[:, 0:512] = w_qkv[:, cs:cs + 512]
        wqk[:, 512:1024] = w_qkv[:, C + cs:C + cs + 512]
        bqk = np.empty((128, 8), dtype=np.float32)
        for j in range(4):
            bqk[:, j] = b_qkv[cs + j * 128: cs + (j + 1) * 128]
            bqk[:, 4 + j] = b_qkv[C + cs + j * 128: C + cs + (j + 1) * 128]
        in_maps.append({
            "xt": np.ascontiguousarray(x[b].T).astype(bf),
            "wqk": wqk,
            "wv": w_qkv[:, 2 * C + cs:2 * C + cs + 512].astype(bf),
            "wo": np.ascontiguousarray(w_out[cs:cs + 512, :]).astype(bf),
            "bqk": bqk,
        })

    _CACHE["in_maps"] = in_maps
    res = run_bass_kernel_spmd(_get_nc(), in_maps, core_ids=list(range(8)))

    # host epilogue: sum head-group partials, add b_out and the V-bias term
    b_eff = b_out + b_qkv[2 * C:3 * C] @ w_out
    out = np.empty((B, T, C), dtype=np.float32)
    for b in range(B):
        acc = (res.results[2 * b]["outT"].astype(np.float32)
               + res.results[2 * b + 1]["outT"].astype(np.float32))
        out[b] = acc.T + b_eff[None, :]
    return out
